# revision 12
# baseline (speedup 1.0000x reference)
"""ChebGCN3Multi fused Trainium2 kernel (8 NeuronCores).

Strategy (graph-partition parallelism, per the sharding hint):
  - Nodes are sharded across the 8 cores (SH = N/8 rows each).
  - The scaled-Laplacian operator S (and its powers S^2, S^3) is densified;
    each core holds the transposed column-slice S^k[:, rows_j] in SBUF and
    computes P_k = S^k @ X for its rows with plain dense matmuls.
  - S^2 and S^3 slices are computed on device from S.
  - Per layer: AllGather of the (transposed) activations, one fused
    "SP-matmul" producing P1,P2,P3, then the Chebyshev weight matmul with
    the K basis folded into pre-combined weights, GraphNorm stats via a tiny
    AllReduce, and a fused affine+LeakyReLU epilogue.
  - The three branches are software-pipelined round-robin so collectives hide
    under the other branches' compute.

All heavy FLOPs run on device in bf16 with fp32 PSUM accumulation.
"""

import math

import numpy as np
import ml_dtypes

import concourse.bass as bass
import concourse.mybir as mybir
from concourse import bacc
import concourse.tile as tile
from concourse.bass_utils import run_bass_kernel_spmd

BF16 = mybir.dt.bfloat16
FP32 = mybir.dt.float32
AFT = mybir.ActivationFunctionType

P = 128
NC = 8
EPS = 1e-5

# Full-size problem constants (hardcoded; harness calls with these shapes).
N_FULL = 4096
HID_FULL = [4096, 2048, 1024, 512]
OUT_FULL = 10


class Cfg:
    def __init__(self, N=N_FULL, HID=None, OUT=OUT_FULL):
        self.N = N
        self.HID = HID or list(HID_FULL)
        self.OUT = OUT
        self.dims = self.HID + [self.HID[0]]  # per-layer fin: dims[i], fout: dims[i+1]
        self.SH = N // NC                     # node shard per core
        self.CT = N // P                      # node tiles
        assert self.SH % P == 0 or self.SH < P or True


def _ceil_div(a, b):
    return (a + b - 1) // b


# --------------------------------------------------------------------------
# Device program
# --------------------------------------------------------------------------

def build_program(cfg: Cfg):
    N, SH, CT, OUT = cfg.N, cfg.SH, cfg.CT, cfg.OUT
    dims = cfg.dims
    nc = bacc.Bacc(None, target_bir_lowering=False, num_devices=NC)

    # ---- I/O ------------------------------------------------------------
    snat = nc.declare_dram_parameter("snat", [N, N], BF16, isOutput=False)
    stj = nc.declare_dram_parameter("stj", [N, SH], BF16, isOutput=False)
    feat = nc.declare_dram_parameter("feat3", [3, N, N], BF16, isOutput=False)
    x0T = nc.declare_dram_parameter("x0T", [3, N, SH], BF16, isOutput=False)
    a_in = [
        nc.declare_dram_parameter(f"a{i}", [3, 4 * dims[i], dims[i + 1]], BF16, isOutput=False)
        for i in range(4)
    ]
    # gparams packed [3, 4, T_i, 128]: (cb, gw, gb, gm) per feature
    gp_in = [
        nc.declare_dram_parameter(f"gp{i}", [3, 4, dims[i + 1] // P, P], FP32, isOutput=False)
        for i in range(4)
    ]
    lw_in = nc.declare_dram_parameter("lw", [3, N, OUT], FP32, isOutput=False)
    lb_in = nc.declare_dram_parameter("lb", [3, OUT], FP32, isOutput=False)
    out_ext = nc.declare_dram_parameter("out", [3, OUT], FP32, isOutput=True)

    # ---- internal DRAM --------------------------------------------------
    xT = [[None] * 5 for _ in range(3)]  # xT[b][i]: [fin_i, SH] transposed acts
    for b in range(3):
        xT[b][0] = x0T  # layer 0 input is the external x0T (slice [b])
        for i in range(1, 4):
            xT[b][i] = nc.dram_tensor(f"xT_{b}_{i}", [dims[i], SH], BF16)
    pstk = [[nc.dram_tensor(f"pstk_{b}_{i}", [3 * dims[i], SH], BF16) for i in range(4)]
            for b in range(3)]
    agout = [[None] * 4 for _ in range(3)]
    for b in range(3):
        for i in range(1, 4):
            agout[b][i] = nc.dram_tensor(f"agout_{b}_{i}", [NC, dims[i], SH], BF16,
                                         addr_space="Shared")
    convraw = [[nc.dram_tensor(f"craw_{b}_{i}", [dims[i + 1], SH], BF16) for i in range(4)]
               for b in range(3)]
    statin = [[nc.dram_tensor(f"stin_{b}_{i}", [2, dims[i + 1] // P, P], FP32) for i in range(4)]
              for b in range(3)]
    statout = [[nc.dram_tensor(f"stout_{b}_{i}", [2, dims[i + 1] // P, P], FP32,
                               addr_space="Shared") for i in range(4)]
               for b in range(3)]
    garin = [nc.dram_tensor(f"garin_{b}", [CT, P], FP32) for b in range(3)]
    garout = [nc.dram_tensor(f"garout_{b}", [CT, P], FP32, addr_space="Shared")
              for b in range(3)]

    RG = [list(range(NC))]

    from contextlib import ExitStack

    with tile.TileContext(nc) as tc, ExitStack() as ctx:
        spj_pool = ctx.enter_context(tc.tile_pool(name="spj", bufs=1))
        slab_pool = ctx.enter_context(tc.tile_pool(name="slab", bufs=CT + 2))
        wrhs_pool = ctx.enter_context(tc.tile_pool(name="wrhs", bufs=3))
        wlhs_pool = ctx.enter_context(tc.tile_pool(name="wlhs", bufs=3))
        ev_pool = ctx.enter_context(tc.tile_pool(name="ev", bufs=3))
        sp_psum = ctx.enter_context(tc.tile_pool(name="sp_psum", bufs=3, space="PSUM"))
        w_psum = ctx.enter_context(tc.tile_pool(name="w_psum", bufs=4, space="PSUM"))
        small_pool = ctx.enter_context(tc.tile_pool(name="small", bufs=4))
        gsum_pool = ctx.enter_context(tc.tile_pool(name="gsum", bufs=3))

        if True:

            # SBUF-resident [S^T | S^2T | S^3T][:, rows_j] : [128, CT, 3*SH]
            spj = spj_pool.tile([P, CT, 3 * SH], BF16)
            for c_t in range(CT):
                nc.sync.dma_start(out=spj[:, c_t, 0:SH], in_=stj[c_t * P:(c_t + 1) * P, :])

            # ---- precompute S^2T, S^3T slices on device -----------------
            # OUTk[c, n] = sum_m snat[m, c] * spj[m, (k-1) block][:, n]
            for k in (1, 2):
                for cg in range(_ceil_div(CT, 4)):
                    nsl = min(4, CT - cg * 4)
                    slabs = []
                    for m_t in range(CT):
                        sl = slab_pool.tile([P, 4 * P], BF16, tag="slab", name="sl")
                        nc.sync.dma_start(
                            out=sl[:, 0:nsl * P],
                            in_=snat[m_t * P:(m_t + 1) * P, cg * 4 * P: cg * 4 * P + nsl * P])
                        slabs.append(sl)
                    for c_sub in range(nsl):
                        c_t = cg * 4 + c_sub
                        ps = sp_psum.tile([P, SH], FP32, name="ps", tag="spps")
                        for m_t in range(CT):
                            nc.tensor.matmul(
                                ps[:, :],
                                lhsT=slabs[m_t][:, c_sub * P:(c_sub + 1) * P],
                                rhs=spj[:, m_t, (k - 1) * SH:k * SH],
                                start=(m_t == 0), stop=(m_t == CT - 1))
                        ev = ev_pool.tile([P, SH], BF16, tag="ev", name="ev")
                        nc.vector.tensor_copy(ev[:, :], ps[:, :])
                        nc.vector.tensor_copy(spj[:, c_t, k * SH:(k + 1) * SH], ev[:, :])

            # ---- head helper state -------------------------------------
            gsum = [gsum_pool.tile([P, CT], FP32, tag="gsum", name="gsum") for _ in range(3)]

            # ---- main pipeline: steps (layer-major, branch round-robin) --
            for i in range(4):
                fin, fout = dims[i], dims[i + 1]
                FT, OT = fin // P, fout // P
                K4 = 4 * fin // P
                for b in range(3):
                    # ---------- SP-matmul: P1,P2,P3 for (b, i) ----------
                    for fg in range(_ceil_div(FT, 4)):
                        nsl = min(4, FT - fg * 4)
                        slabs = []
                        for c_t in range(CT):
                            sl = slab_pool.tile([P, 4 * P], BF16, tag="slab", name="sl")
                            if i == 0:
                                nc.sync.dma_start(
                                    out=sl[:, 0:nsl * P],
                                    in_=feat[b, c_t * P:(c_t + 1) * P,
                                             fg * 4 * P: fg * 4 * P + nsl * P])
                            else:
                                # gathered transposed shards: agout[r, f, n_l]
                                r = (c_t * P) // SH
                                n_off = (c_t * P) % SH
                                nc.sync.dma_start(
                                    out=sl[:, 0:nsl * P],
                                    in_=agout[b][i][r,
                                                    fg * 4 * P: fg * 4 * P + nsl * P,
                                                    n_off:n_off + P],
                                    transpose=True)
                            slabs.append(sl)
                        for f_sub in range(nsl):
                            f_t = fg * 4 + f_sub
                            pss = [sp_psum.tile([P, SH], FP32, name="spps", tag="spps") for _ in range(3)]
                            for c_t in range(CT):
                                for k in range(3):
                                    nc.tensor.matmul(
                                        pss[k][:, :],
                                        lhsT=slabs[c_t][:, f_sub * P:(f_sub + 1) * P],
                                        rhs=spj[:, c_t, k * SH:(k + 1) * SH],
                                        start=(c_t == 0), stop=(c_t == CT - 1))
                            for k in range(3):
                                ev = ev_pool.tile([P, SH], BF16, tag="ev", name="ev")
                                nc.vector.tensor_copy(ev[:, :], pss[k][:, :])
                                nc.sync.dma_start(
                                    out=pstk[b][i][k * fin + f_t * P: k * fin + (f_t + 1) * P, :],
                                    in_=ev[:, :])

                    # ---------- W-matmul + stats for (b, i) -------------
                    ssum = small_pool.tile([P, OT], FP32, tag="ssum")
                    ssq = small_pool.tile([P, OT], FP32, tag="ssq")
                    for ch in range(_ceil_div(fout, 512)):
                        nj = min(4, OT - ch * 4)
                        pss = [w_psum.tile([P, SH], FP32, name="wps", tag="wps") for _ in range(nj)]
                        for k_t in range(K4):
                            rhs_t = wrhs_pool.tile([P, SH], BF16, tag="wrhs")
                            if k_t < FT:
                                if i == 0:
                                    src = x0T[b, k_t * P:(k_t + 1) * P, :]
                                else:
                                    src = xT[b][i][k_t * P:(k_t + 1) * P, :]
                            else:
                                src = pstk[b][i][(k_t - FT) * P + 0:(k_t - FT) * P + P, :]
                            nc.sync.dma_start(out=rhs_t[:, :], in_=src)
                            a_sl = wlhs_pool.tile([P, 4 * P], BF16, tag="wlhs")
                            nc.sync.dma_start(
                                out=a_sl[:, 0:nj * P],
                                in_=a_in[i][b, k_t * P:(k_t + 1) * P,
                                            ch * 512: ch * 512 + nj * P])
                            for j in range(nj):
                                nc.tensor.matmul(
                                    pss[j][:, :],
                                    lhsT=a_sl[:, j * P:(j + 1) * P],
                                    rhs=rhs_t[:, :],
                                    start=(k_t == 0), stop=(k_t == K4 - 1))
                        for j in range(nj):
                            o_t = ch * 4 + j
                            ev = ev_pool.tile([P, SH], BF16, tag="ev", name="ev")
                            nc.scalar.activation(ev[:, :], pss[j][:, :], AFT.Copy,
                                                 accum_out=ssum[:, o_t:o_t + 1])
                            sq = ev_pool.tile([P, SH], BF16, tag="sq")
                            nc.scalar.activation(sq[:, :], pss[j][:, :], AFT.Square,
                                                 accum_out=ssq[:, o_t:o_t + 1])
                            nc.sync.dma_start(
                                out=convraw[b][i][o_t * P:(o_t + 1) * P, :], in_=ev[:, :])
                    # stats -> DRAM -> AllReduce
                    nc.sync.dma_start(
                        out=statin[b][i][0].rearrange("t p -> p t"), in_=ssum[:, :])
                    nc.sync.dma_start(
                        out=statin[b][i][1].rearrange("t p -> p t"), in_=ssq[:, :])
                    nc.gpsimd.collective_compute(
                        "AllReduce", mybir.AluOpType.add, replica_groups=RG,
                        ins=[statin[b][i].ap().opt()], outs=[statout[b][i].ap().opt()])

                    # ---------- affine params ---------------------------
                    st_s = small_pool.tile([P, OT], FP32, tag="sts")
                    st_q = small_pool.tile([P, OT], FP32, tag="stq")
                    nc.sync.dma_start(out=st_s[:, :], in_=statout[b][i][0].rearrange("t p -> p t"))
                    nc.sync.dma_start(out=st_q[:, :], in_=statout[b][i][1].rearrange("t p -> p t"))
                    gp = small_pool.tile([P, 4 * OT], FP32, tag="gp")
                    nc.sync.dma_start(out=gp[:, :], in_=gp_in[i][b].rearrange("g t p -> p (g t)"))
                    cb = gp[:, 0 * OT:1 * OT]
                    gw = gp[:, 1 * OT:2 * OT]
                    gb = gp[:, 2 * OT:3 * OT]
                    gm = gp[:, 3 * OT:4 * OT]
                    mean_c = small_pool.tile([P, OT], FP32, tag="mean")
                    ex2 = small_pool.tile([P, OT], FP32, tag="ex2")
                    nc.vector.tensor_scalar_mul(mean_c[:, :], st_s[:, :], 1.0 / N)
                    nc.vector.tensor_scalar_mul(ex2[:, :], st_q[:, :], 1.0 / N)
                    M = small_pool.tile([P, OT], FP32, tag="M")
                    nc.vector.tensor_add(M[:, :], mean_c[:, :], cb)
                    ctr = small_pool.tile([P, OT], FP32, tag="ctr")
                    nc.vector.tensor_mul(ctr[:, :], M[:, :], gm)
                    # var = ex2 + 2*cb*mean_c + cb^2 - 2*ctr*M + ctr^2
                    t1 = small_pool.tile([P, OT], FP32, tag="t1")
                    var = small_pool.tile([P, OT], FP32, tag="var")
                    nc.vector.tensor_mul(t1[:, :], cb, mean_c[:, :])
                    nc.vector.tensor_scalar_mul(t1[:, :], t1[:, :], 2.0)
                    nc.vector.tensor_add(var[:, :], ex2[:, :], t1[:, :])
                    nc.vector.tensor_mul(t1[:, :], cb, cb)
                    nc.vector.tensor_add(var[:, :], var[:, :], t1[:, :])
                    nc.vector.tensor_mul(t1[:, :], ctr[:, :], M[:, :])
                    nc.vector.tensor_scalar_mul(t1[:, :], t1[:, :], -2.0)
                    nc.vector.tensor_add(var[:, :], var[:, :], t1[:, :])
                    nc.vector.tensor_mul(t1[:, :], ctr[:, :], ctr[:, :])
                    nc.vector.tensor_add(var[:, :], var[:, :], t1[:, :])
                    std = small_pool.tile([P, OT], FP32, tag="std")
                    nc.vector.tensor_scalar_add(std[:, :], var[:, :], float(EPS))
                    nc.scalar.activation(std[:, :], std[:, :], AFT.Sqrt)
                    rstd = small_pool.tile([P, OT], FP32, tag="rstd")
                    nc.vector.reciprocal(rstd[:, :], std[:, :])
                    scale = small_pool.tile([P, OT], FP32, tag="scale")
                    shift = small_pool.tile([P, OT], FP32, tag="shift")
                    nc.vector.tensor_mul(scale[:, :], gw, rstd[:, :])
                    nc.vector.tensor_sub(t1[:, :], cb, ctr[:, :])
                    nc.vector.tensor_mul(shift[:, :], t1[:, :], scale[:, :])
                    nc.vector.tensor_add(shift[:, :], shift[:, :], gb)

                    # ---------- affine apply ----------------------------
                    for o_t in range(OT):
                        cr = ev_pool.tile([P, SH], BF16, tag="cr")
                        nc.sync.dma_start(out=cr[:, :],
                                          in_=convraw[b][i][o_t * P:(o_t + 1) * P, :])
                        if i < 3:
                            z = ev_pool.tile([P, SH], BF16, tag="z")
                            nc.scalar.activation(z[:, :], cr[:, :], AFT.Identity,
                                                 bias=shift[:, o_t:o_t + 1],
                                                 scale=scale[:, o_t:o_t + 1])
                            z2 = ev_pool.tile([P, SH], BF16, tag="z2")
                            nc.vector.tensor_scalar_mul(z2[:, :], z[:, :], 0.1)
                            y = ev_pool.tile([P, SH], BF16, tag="y")
                            nc.vector.tensor_max(y[:, :], z[:, :], z2[:, :])
                            nc.sync.dma_start(out=xT[b][i + 1][o_t * P:(o_t + 1) * P, :],
                                              in_=y[:, :])
                        else:
                            y = ev_pool.tile([P, SH], FP32, tag="yf")
                            nc.scalar.activation(y[:, :], cr[:, :], AFT.Identity,
                                                 bias=shift[:, o_t:o_t + 1],
                                                 scale=scale[:, o_t:o_t + 1])
                            x0t = ev_pool.tile([P, SH], BF16, tag="x0t")
                            nc.sync.dma_start(out=x0t[:, :],
                                              in_=x0T[b, o_t * P:(o_t + 1) * P, :])
                            y2 = ev_pool.tile([P, SH], FP32, tag="y2")
                            nc.vector.tensor_add(y2[:, :], y[:, :], x0t[:, :])
                            yr = ev_pool.tile([P, SH], FP32, tag="yr")
                            nc.scalar.activation(yr[:, :], y2[:, :], AFT.Relu,
                                                 accum_out=gsum[b][:, o_t:o_t + 1])
                    if i < 3:
                        nc.gpsimd.collective_compute(
                            "AllGather", mybir.AluOpType.bypass, replica_groups=RG,
                            ins=[xT[b][i + 1].ap().opt()],
                            outs=[agout[b][i + 1].ap().opt()])
                    else:
                        nc.sync.dma_start(out=garin[b].rearrange("t p -> p t"),
                                          in_=gsum[b][:, :])
                        nc.gpsimd.collective_compute(
                            "AllReduce", mybir.AluOpType.add, replica_groups=RG,
                            ins=[garin[b].ap().opt()], outs=[garout[b].ap().opt()])

            # ---- heads -------------------------------------------------
            for b in range(3):
                g = small_pool.tile([P, CT], FP32, tag="g")
                nc.sync.dma_start(out=g[:, :], in_=garout[b].rearrange("t p -> p t"))
                gr = small_pool.tile([P, CT], FP32, tag="gr")
                # g = relu(mean) ; mean = sum / N
                nc.vector.tensor_scalar_mul(gr[:, :], g[:, :], 1.0 / N)
                nc.scalar.activation(gr[:, :], gr[:, :], AFT.Relu)
                lwt = small_pool.tile([P, CT * OUT], FP32, tag="lwt")
                for c_t in range(CT):
                    nc.sync.dma_start(
                        out=lwt[:, c_t * OUT:(c_t + 1) * OUT],
                        in_=lw_in[b, c_t * P:(c_t + 1) * P, :])
                po = w_psum.tile([P, SH], FP32, tag="wps", name="po")
                for c_t in range(CT):
                    nc.tensor.matmul(
                        po[0:1, 0:OUT],
                        lhsT=gr[:, c_t:c_t + 1],
                        rhs=lwt[:, c_t * OUT:(c_t + 1) * OUT],
                        start=(c_t == 0), stop=(c_t == CT - 1))
                o = small_pool.tile([1, OUT], FP32, tag="o")
                lbt = small_pool.tile([1, OUT], FP32, tag="lbt")
                nc.sync.dma_start(out=lbt[:, :], in_=lb_in[b:b + 1, :])
                nc.vector.tensor_add(o[:, :], po[0:1, 0:OUT], lbt[:, :])
                if b == 0:
                    # softplus = ln(1 + exp(o)) (Softplus has no act table here)
                    o2 = small_pool.tile([1, OUT], FP32, tag="o2")
                    nc.scalar.activation(o2[:, :], o[:, :], AFT.Exp)
                    nc.vector.tensor_scalar_add(o2[:, :], o2[:, :], 1.0)
                    nc.scalar.activation(o2[:, :], o2[:, :], AFT.Ln)
                    o = o2
                mx = small_pool.tile([1, 1], FP32, tag="mx")
                nc.vector.reduce_max(mx[:, :], o[:, :], axis=mybir.AxisListType.X)
                nmx = small_pool.tile([1, 1], FP32, tag="nmx")
                nc.vector.tensor_scalar_mul(nmx[:, :], mx[:, :], -1.0)
                e = small_pool.tile([1, OUT], FP32, tag="e")
                nc.scalar.activation(e[:, :], o[:, :], AFT.Exp, bias=nmx[:, 0:1])
                se = small_pool.tile([1, 1], FP32, tag="se")
                nc.vector.reduce_sum(se[:, :], e[:, :], axis=mybir.AxisListType.X)
                rse = small_pool.tile([1, 1], FP32, tag="rse")
                nc.vector.reciprocal(rse[:, :], se[:, :])
                sm = small_pool.tile([1, OUT], FP32, tag="sm")
                nc.vector.tensor_scalar_mul(sm[:, :], e[:, :], rse[:, 0:1])
                ro = small_pool.tile([1, OUT], FP32, tag="ro")
                nc.scalar.activation(ro[:, :], o[:, :], AFT.Relu)
                fin_t = small_pool.tile([1, OUT], FP32, tag="fin")
                nc.vector.tensor_mul(fin_t[:, :], sm[:, :], ro[:, :])
                nc.sync.dma_start(out=out_ext[b:b + 1, :], in_=fin_t[:, :])

    nc.finalize()
    return nc


# --------------------------------------------------------------------------
# Host-side prep
# --------------------------------------------------------------------------

def host_prep(edge_index, feats, params, cfg: Cfg):
    """Build per-core input maps. feats: list of 3 [N,N] fp32 arrays."""
    N, SH, OUT = cfg.N, cfg.SH, cfg.OUT
    dims = cfg.dims
    row = np.asarray(edge_index[0])
    col = np.asarray(edge_index[1])
    deg = np.bincount(row, minlength=N).astype(np.float32)
    dinv = np.where(deg > 0, 1.0 / np.sqrt(np.maximum(deg, 1.0)), 0.0).astype(np.float32)
    norm = (-dinv[row] * dinv[col]).astype(np.float32)
    St = np.zeros((N, N), dtype=np.float32)  # St[c, r] = S[r, c]
    np.add.at(St, (col, row), norm)
    bf = ml_dtypes.bfloat16
    St_bf = St.astype(bf)
    Snat_bf = St_bf.T.copy()  # S natural [r, c] (same rounded values)

    feat3 = np.stack([np.asarray(f, np.float32) for f in feats]).astype(bf)  # [3,N,N]
    x0T_full = np.ascontiguousarray(np.transpose(feat3, (0, 2, 1)))  # [3,N,N] (f, node)

    a_list, gp_list = [], []
    for i in range(4):
        fin, fout = dims[i], dims[i + 1]
        A = np.zeros((3, 4 * fin, fout), dtype=np.float32)
        GP = np.zeros((3, 4, fout // P, P), dtype=np.float32)
        for b, key in enumerate(["b1", "b2", "b3"]):
            p = params[key]
            W = np.asarray(p["W"][i], np.float32)  # [4, fin, fout]
            A[b, 0 * fin:1 * fin] = W[0] - W[2]
            A[b, 1 * fin:2 * fin] = W[1] - 3.0 * W[3]
            A[b, 2 * fin:3 * fin] = 2.0 * W[2]
            A[b, 3 * fin:4 * fin] = 4.0 * W[3]
            GP[b, 0] = np.asarray(p["cb"][i], np.float32).reshape(fout // P, P)
            GP[b, 1] = np.asarray(p["gw"][i], np.float32).reshape(fout // P, P)
            GP[b, 2] = np.asarray(p["gb"][i], np.float32).reshape(fout // P, P)
            GP[b, 3] = np.asarray(p["gm"][i], np.float32).reshape(fout // P, P)
        a_list.append(A.astype(bf))
        gp_list.append(GP)
    lw = np.stack([np.asarray(params[k]["lw"], np.float32) for k in ["b1", "b2", "b3"]])
    lb = np.stack([np.asarray(params[k]["lb"], np.float32) for k in ["b1", "b2", "b3"]])

    in_maps = []
    for j in range(NC):
        rows = slice(j * SH, (j + 1) * SH)
        m = {
            "snat": Snat_bf,
            "stj": np.ascontiguousarray(St_bf[:, rows]),
            "feat3": feat3,
            "x0T": np.ascontiguousarray(x0T_full[:, :, rows]),
            "lw": lw,
            "lb": lb,
        }
        for i in range(4):
            m[f"a{i}"] = a_list[i]
            m[f"gp{i}"] = gp_list[i]
        in_maps.append(m)
    return in_maps


# --------------------------------------------------------------------------
# Entry point
# --------------------------------------------------------------------------

_CACHE = {}


def _get_program(cfg_key):
    if cfg_key not in _CACHE:
        _CACHE[cfg_key] = build_program(Cfg())
    return _CACHE[cfg_key]


def kernel(edge_index, feat, feat_1, feat_2, params):
    cfg = Cfg()
    assert feat.shape == (cfg.N, cfg.N)
    in_maps = host_prep(edge_index, [feat, feat_1, feat_2], params, cfg)
    nc = _get_program("full")
    res = run_bass_kernel_spmd(nc, in_maps, core_ids=list(range(NC)))
    out = np.asarray(res.results[0]["out"], np.float32)
    return (out[0].copy(), out[1].copy(), out[2].copy())


if __name__ == "__main__":
    import reference

    inputs = reference.setup_inputs()
    outs = kernel(**{k: np.asarray(v) if not isinstance(v, dict) else v
                     for k, v in inputs.items()})
    exp = reference.reference(**inputs)
    for a, b in zip(outs, exp):
        print(np.asarray(b))
        print(a)
